# revision 3
# baseline (speedup 1.0000x reference)
"""Trainium2 Bass kernel for a dense transformer block (B=2, T=2048, D=2048,
N=16 q heads, K=8 kv heads, H=128, F=8192, causal attention, RoPE, RMSNorm,
GeGLU FFN), sharded over 8 NeuronCores.

Same sharding as the original baseline (2 q heads + 1 kv head per core,
Megatron-style column-split QKV / row-split attn_vec, data-parallel FFN on
the core's 512-token slice, one AllToAll), rebuilt for throughput:
  - pre-tiled, partition-major host layouts so every weight/activation load
    is ONE dma_start with large contiguous runs (~70 DMAs vs 937: far less
    HWDGE/SEQ serialization);
  - causal diagonal blocks computed at partial width (no wasted matmul
    columns, no memsets);
  - softmax denominators accumulated on the Vector engine (two alternating
    bf16 accumulators), one ones-matmul per query block instead of one per
    kv block; each block's normalize is deferred one block so the DVE add
    chain hides behind the next block's matmuls on the in-order PE;
  - residual kept in SBUF (bf16) instead of a DRAM round-trip;
  - attn_vec weights prefetched during attention; FFN weights double-
    buffered with half-tile loads; DMA prefetches ordered on the SP queue
    (or pushed to the idle Activation queue) so no sem-waiting DMA
    head-of-line-blocks a later-needed one.
"""
import numpy as np

import concourse.bass as bass
import concourse.bacc as bacc
import concourse.tile as tile
from concourse import mybir
from concourse.bass_utils import run_bass_kernel_spmd
from concourse.masks import make_identity, make_upper_triangular

F32 = mybir.dt.float32
BF16 = mybir.dt.bfloat16
NP_BF16 = mybir.dt.np(BF16)

B, T, D = 2, 2048, 2048
N, K, H = 16, 8, 128
F = 8192
NCORES = 8
BT = B * T                  # 4096 flattened tokens (tok = b*T + t)
TOKC = BT // NCORES         # 512 tokens per chunk / per-core slice
NCHUNK = NCORES             # 8 token chunks
DT_TILES = D // 128         # 16
F_TILES = F // 128          # 64
GH = N // K                 # 2 q heads per kv head (= per core)
QB = T // TOKC              # 4 query chunks per batch
KVB = T // 128              # 16 kv blocks per batch
EPS = 1e-6


def build_program(n_cores=NCORES, sim=False, reps=1, no_cc=False):
    nc = bacc.Bacc("TRN2", target_bir_lowering=False, debug=False,
                   num_devices=n_cores)

    # ---- I/O (all pre-tiled, partition-major) ----
    xt = nc.dram_tensor("xt", [NCHUNK, 128, DT_TILES, TOKC], BF16,
                        kind="ExternalInput")
    xs = nc.dram_tensor("xs", [128, DT_TILES, TOKC], F32,
                        kind="ExternalInput")
    wqkv = nc.dram_tensor("wqkv", [128, DT_TILES, 4 * H], BF16,
                          kind="ExternalInput")
    wav = nc.dram_tensor("wav", [128, DT_TILES, D], BF16,
                         kind="ExternalInput")
    wg = nc.dram_tensor("wg", [F // 512, 128, DT_TILES, 1024], BF16,
                        kind="ExternalInput")
    wlin = nc.dram_tensor("wlin", [2, 4, 128, 16, 1024], BF16,
                          kind="ExternalInput")
    costab = nc.dram_tensor("costab", [64, BT], F32, kind="ExternalInput")
    sintab = nc.dram_tensor("sintab", [64, BT], F32, kind="ExternalInput")
    outT = nc.dram_tensor("outT", [DT_TILES, 128, TOKC], F32,
                          kind="ExternalOutput")

    with tile.TileContext(nc) as tc:
        for _ in range(reps):
            _build(tc, (1 if no_cc else n_cores), sim, xt, xs, wqkv, wav,
                   wg, wlin, costab, sintab, outT)
    nc.compile()
    return nc


def _build(tc, n_cores, sim, xt, xs, wqkv, wav, wg, wlin, costab, sintab,
           outT):
    nc = tc.nc
    AF = mybir.ActivationFunctionType

    with tc.tile_pool(name="const", bufs=1) as const, \
         tc.tile_pool(name="dram", bufs=1, space="DRAM") as dram:
        ones128 = const.tile([128, 1], BF16, tag="ones128", name="ones128")
        nc.vector.memset(ones128[:], 1.0)
        ones_row = const.tile([1, 128], F32, tag="ones_row", name="ones_row")
        nc.vector.memset(ones_row[:], 1.0)
        ident = const.tile([128, 128], BF16, tag="ident", name="ident")
        make_identity(nc, ident[:])
        # keep-mask for diagonal attention blocks on S^T [kv, q]:
        # U[p, f] = 1 if p <= f else 0
        umask = const.tile([128, 128], BF16, tag="umask", name="umask")
        make_upper_triangular(nc, umask[:], val=1.0, diag=True)
        eps1 = const.tile([1, 1], F32, tag="eps1", name="eps1")
        nc.vector.memset(eps1[:], EPS)

        # DRAM buffers for the single AllToAll (both heads)
        cc_in = dram.tile([NCHUNK, GH, 128, TOKC], BF16, tag="cc_in",
                          name="cc_in")
        cc_out = dram.tile([NCHUNK, GH, 128, TOKC], BF16, tag="cc_out",
                           name="cc_out")

        # residual/normed tiles live to the very end -> outermost pool
        pRes_cm = tc.tile_pool(name="pRes", bufs=1)
        pRes = pRes_cm.__enter__()
        pers_cm = tc.tile_pool(name="persAct", bufs=1)
        pers = pers_cm.__enter__()
        qT = [pers.tile([128, BT], BF16, tag=f"qT{h}", name=f"qT{h}")
              for h in range(GH)]
        kT = pers.tile([128, BT], BF16, tag="kT", name="kT")
        vtok = [pers.tile([128, H], BF16, tag=f"vtok{g}", name=f"vtok{g}")
                for g in range(2 * KVB)]
        encT = [pers.tile([128, BT], BF16, tag=f"encT{h}", name=f"encT{h}")
                for h in range(GH)]

        _phase_proj(tc, nc, AF, n_cores, xt, wqkv, costab, sintab,
                    ones128, ones_row, ident, eps1, qT, kT, vtok)
        # pool spanning attention -> attn_vec/stats (closed before the FFN
        # weight streams): attn_vec weights, gathered enc
        pAV_cm = tc.tile_pool(name="pAV2", bufs=1)
        pAV = pAV_cm.__enter__()
        wav_t, encf = _phase_attn(tc, nc, AF, n_cores, pAV, wav,
                                  ones128, ones_row, umask, qT, kT,
                                  vtok, encT, cc_in, cc_out)
        _phase_ffn(tc, nc, AF, sim, pAV_cm, pers_cm, pRes, xs, wav_t, encf,
                   wg, wlin, ones128, ones_row, eps1, outT)
        pRes_cm.__exit__(None, None, None)


def _phase_proj(tc, nc, AF, n_cores, xt, wqkv, costab, sintab,
                ones128, ones_row, ident, eps1, qT, kT, vtok):
    """Local RMSNorm stats (per chunk), QKV projection + RoPE."""
    with tc.tile_pool(name="pSt", bufs=1) as pSt, \
         tc.tile_pool(name="psA", bufs=1, space="PSUM") as psA:
        # resident qkv weights: one load [128, 16, 512]
        wq_t = pSt.tile([128, DT_TILES, 4 * H], BF16, tag="wq_t", name="wq_t")
        nc.sync.dma_start(wq_t[:], wqkv[:])

        _proj_chunks(tc, nc, AF, xt, costab, sintab, wq_t, ones128,
                     eps1, psA, ones_row, ident, qT, kT, vtok)


def _proj_chunks(tc, nc, AF, xt, costab, sintab, wq_t, ones128,
                 eps1, psA, ones_row, ident, qT, kT, vtok):
    with tc.tile_pool(name="pX", bufs=2) as pX, \
         tc.tile_pool(name="pR", bufs=2) as pR, \
         tc.tile_pool(name="pT", bufs=2) as pT:
        def load_chunk(c):
            cols = slice(c * TOKC, (c + 1) * TOKC)
            xc = pX.tile([128, DT_TILES, TOKC], BF16, tag="xc", name="xc")
            nc.sync.dma_start(xc[:], xt[c])
            cos_c = pT.tile([64, TOKC], F32, tag="cos_c", name="cos_c")
            nc.sync.dma_start(cos_c[:], costab[:, cols])
            sin_c = pT.tile([64, TOKC], F32, tag="sin_c", name="sin_c")
            nc.sync.dma_start(sin_c[:], sintab[:, cols])
            return xc, cos_c, sin_c

        pending = load_chunk(0)
        for c in range(NCHUNK):
            cols = slice(c * TOKC, (c + 1) * TOKC)
            xc, cos_c, sin_c = pending
            if c + 1 < NCHUNK:
                pending = load_chunk(c + 1)

            # RMSNorm stats for this chunk: ssq = sum_D x^2 (bf16 x)
            ssq = psA.tile([1, TOKC], F32, tag="ssq", name="ssq")
            for kt in range(DT_TILES):
                sq = pR.tile([128, TOKC], BF16, tag="sq", name="sq")
                nc.vector.tensor_mul(sq[:], xc[:, kt, :], xc[:, kt, :])
                nc.tensor.matmul(ssq[:], ones128[:], sq[:],
                                 start=(kt == 0), stop=(kt == DT_TILES - 1))
            sd = pR.tile([1, TOKC], F32, tag="sd", name="sd")
            nc.scalar.activation(sd[:], ssq[:], AF.Sqrt, bias=eps1[:],
                                 scale=1.0 / D)
            rr = pR.tile([1, TOKC], F32, tag="rr", name="rr")
            nc.vector.reciprocal(rr[:], sd[:])

            # qkv projection: 4 psum tiles [128, TOKC]
            proj = [psA.tile([128, TOKC], F32, tag=f"proj{ft}",
                             name=f"proj{ft}") for ft in range(4)]
            for kt in range(DT_TILES):
                for ft in range(4):
                    nc.tensor.matmul(
                        proj[ft][:],
                        wq_t[:, kt, ft * 128:(ft + 1) * 128],
                        xc[:, kt, :],
                        start=(kt == 0), stop=(kt == DT_TILES - 1))

            # broadcast r for this chunk to 128 partitions
            rb_ps = psA.tile([128, TOKC], F32, tag="rb_ps", name="rb_ps")
            nc.tensor.matmul(rb_ps[:], ones_row[:], rr[:],
                             start=True, stop=True)
            rb = pR.tile([128, TOKC], F32, tag="rb", name="rb")
            nc.vector.tensor_copy(rb[:], rb_ps[:])
            # r-folded rope tables for this chunk
            cosr = pR.tile([64, TOKC], F32, tag="cosr", name="cosr")
            nc.vector.tensor_mul(cosr[:], cos_c[:], rb[0:64, :])
            sinr = pR.tile([64, TOKC], F32, tag="sinr", name="sinr")
            nc.vector.tensor_mul(sinr[:], sin_c[:], rb[0:64, :])

            # v first (plain r scaling + transpose to token-major) so the PE
            # transposes don't sit behind the whole rope DVE chain
            vsb = pR.tile([128, TOKC], BF16, tag="vsb", name="vsb")
            nc.vector.tensor_mul(vsb[:], proj[3][:], rb[:])
            for j in range(TOKC // 128):
                vt_ps = psA.tile([128, 128], BF16, tag="vt_ps", name="vt_ps")
                nc.tensor.transpose(vt_ps[:], vsb[:, j * 128:(j + 1) * 128],
                                    ident[:])
                nc.vector.tensor_copy(vtok[c * 4 + j][:], vt_ps[:])

            # rope for q heads and k (r folded into the tables)
            for ft in range(3):
                dst = qT[ft] if ft < GH else kT
                ps = proj[ft]
                t1 = pR.tile([64, TOKC], F32, tag="t1", name="t1")
                t2 = pR.tile([64, TOKC], F32, tag="t2", name="t2")
                nc.vector.tensor_mul(t1[:], ps[0:64, :], cosr[:])
                nc.vector.tensor_mul(t2[:], ps[64:128, :], sinr[:])
                nc.vector.tensor_sub(dst[0:64, cols], t1[:], t2[:])
                t3 = pR.tile([64, TOKC], F32, tag="t1", name="t3")
                t4 = pR.tile([64, TOKC], F32, tag="t2", name="t4")
                nc.vector.tensor_mul(t3[:], ps[64:128, :], cosr[:])
                nc.vector.tensor_mul(t4[:], ps[0:64, :], sinr[:])
                nc.vector.tensor_add(dst[64:128, cols], t3[:], t4[:])


def _phase_attn(tc, nc, AF, n_cores, pAV, wav, ones128, ones_row, umask,
                qT, kT, vtok, encT, cc_in, cc_out):
    """Causal attention (S^T formulation), h-outer so each head's enc can
    AllToAll while the next head (or attn_vec) computes.  Also prefetches
    the attn_vec weights and gathers enc per head as its A2A lands."""
    # prefetch attn_vec weights during attention (SP queue, no deps ->
    # transfer overlaps the h=0 attention compute)
    wav_t = pAV.tile([128, DT_TILES, D], BF16, tag="wav_t", name="wav_t")
    nc.sync.dma_start(wav_t[:], wav[:])
    encf = []

    def ship_all():
        # ship both heads' enc in ONE AllToAll: after it, core c holds all
        # 16 heads for its own chunk
        for h in range(GH):
            nc.sync.dma_start(
                cc_in[:, h].rearrange("cq p n -> p cq n"),
                encT[h][:].rearrange("p (cq n) -> p cq n", cq=NCHUNK))
        if n_cores > 1:
            nc.gpsimd.collective_compute(
                "AllToAll", mybir.AluOpType.bypass,
                replica_groups=[list(range(n_cores))],
                ins=[cc_in.opt()], outs=[cc_out.opt()])
        else:
            nc.sync.dma_start(cc_out[:], cc_in[:])
        for h in range(GH):
            ef = pAV.tile([128, NCHUNK, TOKC], BF16, tag=f"encf{h}",
                          name=f"encf{h}")
            nc.sync.dma_start(ef[:],
                              cc_out[:, h].rearrange("j p n -> p j n"))
            encf.append(ef)

    with tc.tile_pool(name="pC", bufs=3) as pC, \
         tc.tile_pool(name="pC2", bufs=2) as pC2, \
         tc.tile_pool(name="psS", bufs=2, space="PSUM") as psS, \
         tc.tile_pool(name="psO", bufs=3, space="PSUM") as psO, \
         tc.tile_pool(name="psD", bufs=1, space="PSUM") as psD:
        finish = None        # deferred normalize of the previous query block
        fin_head = None      # head that finish() completes (if h-tail block)
        for h in range(GH):
            for b in range(B):
                for c in range(QB):
                    cq = b * QB + c
                    q0 = cq * TOKC
                    o_ps = psO.tile([128, TOKC], F32, tag="o_ps", name="o_ps")
                    # two DVE denominator accumulators to halve the serial
                    # add chain (bf16)
                    acc = [pC2.tile([128, TOKC], BF16, tag=f"acc{a}",
                                    name=f"acc{a}") for a in range(2)]
                    nkv = 4 * c + 4   # kv blocks 0 .. 4c+3
                    for j in range(nkv):
                        g = b * KVB + j
                        d = j - 4 * c
                        col0 = d * 128 if d > 0 else 0
                        s_ps = psS.tile([128, TOKC], F32, tag="s_ps",
                                        name="s_ps")
                        nc.tensor.matmul(
                            s_ps[:, col0:], kT[:, g * 128:(g + 1) * 128],
                            qT[h][:, q0 + col0:q0 + TOKC],
                            start=True, stop=True)
                        p_sb = pC.tile([128, TOKC], BF16, tag="p_sb",
                                       name="p_sb")
                        nc.scalar.activation(p_sb[:, col0:], s_ps[:, col0:],
                                             AF.Exp)
                        if d >= 0:
                            # triangular keep-mask on the diagonal 128 cols
                            nc.vector.tensor_mul(
                                p_sb[:, col0:col0 + 128],
                                p_sb[:, col0:col0 + 128], umask[:])
                        # denominator accumulation on DVE (bf16)
                        if j < 2:
                            if col0 > 0:
                                # diagonal first block: left cols unwritten
                                nc.vector.memset(acc[j][:, 0:col0], 0.0)
                            nc.vector.tensor_copy(acc[j][:, col0:],
                                                  p_sb[:, col0:])
                        else:
                            a = acc[j % 2]
                            nc.vector.tensor_add(a[:, col0:], a[:, col0:],
                                                 p_sb[:, col0:])
                        nc.tensor.matmul(o_ps[:, col0:], vtok[g][:],
                                         p_sb[:, col0:],
                                         start=(j == 0), stop=(j == nkv - 1),
                                         skip_group_check=True)
                        if j == 1 and finish is not None:
                            # normalize the PREVIOUS block here so its DVE
                            # denominator chain hides behind this block's
                            # matmuls instead of stalling the in-order PE
                            finish()
                            finish = None

                    def finish(h=h, q0=q0, o_ps=o_ps, acc=acc):
                        # den = column sums of acc0+acc1; enc = O / den
                        den_ps = psD.tile([1, TOKC], F32, tag="den_ps",
                                          name="den_ps")
                        nc.tensor.matmul(den_ps[:], ones128[:], acc[0][:],
                                         start=True, stop=False)
                        nc.tensor.matmul(den_ps[:], ones128[:], acc[1][:],
                                         start=False, stop=True)
                        rec = pC2.tile([1, TOKC], F32, tag="rec", name="rec")
                        nc.vector.reciprocal(rec[:], den_ps[:])
                        db_ps = psD.tile([128, TOKC], F32, tag="db_ps",
                                         name="db_ps")
                        nc.tensor.matmul(db_ps[:], ones_row[:], rec[:],
                                         start=True, stop=True)
                        db = pC2.tile([128, TOKC], F32, tag="db", name="db")
                        nc.vector.tensor_copy(db[:], db_ps[:])
                        nc.vector.tensor_mul(encT[h][:, q0:q0 + TOKC],
                                             o_ps[:], db[:])
                    fin_head = h if (b == B - 1 and c == QB - 1) else None
        finish()
        ship_all()
    return wav_t, encf


def _phase_ffn(tc, nc, AF, sim, pAV_cm, pers_cm, pRes, xs, wav_t, encf,
               wg, wlin, ones128, ones_row, eps1, outT):
    """attn_vec (even/odd kt halves to hide the 2nd collective), residual +
    RMSNorm + GeGLU FFN on this core's 512-token slice."""
    res = [pRes.tile([128, TOKC], BF16, tag=f"res{dt}", name=f"res{dt}")
           for dt in range(DT_TILES)]
    hn = [pRes.tile([128, TOKC], BF16, tag=f"hn{dt}", name=f"hn{dt}")
          for dt in range(DT_TILES)]
    with tc.tile_pool(name="pE", bufs=2) as pE, \
         tc.tile_pool(name="pE3", bufs=3) as pE3, \
         tc.tile_pool(name="pXL", bufs=3) as pXL, \
         tc.tile_pool(name="psE", bufs=1, space="PSUM") as psE, \
         tc.tile_pool(name="psAV", bufs=2, space="PSUM") as psAV:
        # attn_vec (all 16 heads) + residual + sum-of-squares for the
        # FFN RMSNorm
        ssq2 = psE.tile([1, TOKC], F32, tag="ssq2", name="ssq2")
        for dt in range(DT_TILES):
            ao_ps = psAV.tile([128, TOKC], F32, tag="ao_ps", name="ao_ps")
            for kt in range(DT_TILES):
                nc.tensor.matmul(
                    ao_ps[:],
                    wav_t[:, kt, dt * 128:(dt + 1) * 128],
                    encf[kt % 2][:, kt // 2, :],
                    start=(kt == 0), stop=(kt == DT_TILES - 1))
            xsl = pXL.tile([128, TOKC], F32, tag="xsl", name="xsl")
            nc.sync.dma_start(xsl[:], xs[:, dt, :])
            nc.vector.tensor_add(res[dt][:], ao_ps[:], xsl[:])
            sq2 = pE3.tile([128, TOKC], BF16, tag="sq2", name="sq2")
            nc.vector.tensor_mul(sq2[:], res[dt][:], res[dt][:])
            nc.tensor.matmul(ssq2[:], ones128[:], sq2[:],
                             start=(dt == 0), stop=(dt == DT_TILES - 1))
        sd2 = pE.tile([1, TOKC], F32, tag="sd2", name="sd2")
        nc.scalar.activation(sd2[:], ssq2[:], AF.Sqrt, bias=eps1[:],
                             scale=1.0 / D)
        rr2 = pE.tile([1, TOKC], F32, tag="rr2", name="rr2")
        nc.vector.reciprocal(rr2[:], sd2[:])
        r2b_ps = psE.tile([128, TOKC], F32, tag="r2b_ps", name="r2b_ps")
        nc.tensor.matmul(r2b_ps[:], ones_row[:], rr2[:],
                         start=True, stop=True)
        r2b = pE.tile([128, TOKC], F32, tag="r2b", name="r2b")
        nc.vector.tensor_copy(r2b[:], r2b_ps[:])
        for dt in range(DT_TILES):
            nc.vector.tensor_mul(hn[dt][:], res[dt][:], r2b[:])
    # free wav/encf + attention persistents before the FFN weight streams
    pAV_cm.__exit__(None, None, None)
    pers_cm.__exit__(None, None, None)

    # gate path: act = gelu_tanh(hn @ wg0) * (hn @ wg1), act kept in SBUF.
    # pL (down-proj weights) opens BEFORE pW so its space is disjoint: the
    # first wlin loads then have no space-reuse dependency on gate weights.
    with tc.tile_pool(name="pActs", bufs=1) as pActs, \
         tc.tile_pool(name="pL", bufs=2) as pL:
        act = [pActs.tile([128, TOKC], BF16, tag=f"act{f}", name=f"act{f}")
               for f in range(F_TILES)]
        with tc.tile_pool(name="pW", bufs=2) as pW, \
             tc.tile_pool(name="pG", bufs=3) as pG, \
             tc.tile_pool(name="psW", bufs=2, space="PSUM") as psW:
            for fs in range(F // 512):
                HK = DT_TILES // 2
                wgs = []
                for i in range(2):
                    wh = pW.tile([128, HK, 1024], BF16, tag=f"wgs{i}",
                                 name=f"wgs{i}")
                    nc.sync.dma_start(wh[:], wg[fs][:, i * HK:(i + 1) * HK, :])
                    wgs.append(wh)
                for fi in range(4):
                    f = fs * 4 + fi
                    g_ps = psW.tile([128, TOKC], F32, tag="g_ps", name="g_ps")
                    u_ps = psW.tile([128, TOKC], F32, tag="u_ps", name="u_ps")
                    for kt in range(DT_TILES):
                        nc.tensor.matmul(
                            g_ps[:],
                            wgs[kt // HK][:, kt % HK,
                                          fi * 128:(fi + 1) * 128],
                            hn[kt][:],
                            start=(kt == 0), stop=(kt == DT_TILES - 1))
                    for kt in range(DT_TILES):
                        nc.tensor.matmul(
                            u_ps[:],
                            wgs[kt // HK][:, kt % HK,
                                          512 + fi * 128:512 + (fi + 1) * 128],
                            hn[kt][:],
                            start=(kt == 0), stop=(kt == DT_TILES - 1))
                    gg = pG.tile([128, TOKC], BF16, tag="gg", name="gg")
                    if not sim:
                        nc.scalar.activation(gg[:], g_ps[:],
                                             AF.Gelu_apprx_tanh)
                    else:
                        # tanh-gelu composite (CoreSim has no Gelu LUT)
                        x2 = pG.tile([128, TOKC], F32, tag="x2", name="x2")
                        nc.vector.tensor_mul(x2[:], g_ps[:], g_ps[:])
                        x3 = pG.tile([128, TOKC], F32, tag="x3", name="x3")
                        nc.vector.tensor_mul(x3[:], x2[:], g_ps[:])
                        inner = pG.tile([128, TOKC], F32, tag="inner",
                                        name="inner")
                        nc.vector.tensor_scalar(inner[:], x3[:], 0.044715,
                                                None, mybir.AluOpType.mult)
                        nc.vector.tensor_add(inner[:], inner[:], g_ps[:])
                        th = pG.tile([128, TOKC], F32, tag="th", name="th")
                        nc.scalar.activation(th[:], inner[:], AF.Tanh,
                                             scale=0.7978845608028654)
                        nc.vector.tensor_scalar(th[:], th[:], 1.0, 0.5,
                                                mybir.AluOpType.add,
                                                mybir.AluOpType.mult)
                        nc.vector.tensor_mul(gg[:], th[:], g_ps[:])
                    nc.vector.tensor_mul(act[f][:], u_ps[:], gg[:])

        # linear: out^T[dt] = sum_f wlin[f, dt-cols].T @ act[f]  (+ residual)
        with tc.tile_pool(name="pOb", bufs=3) as pOb, \
             tc.tile_pool(name="psL", bufs=1, space="PSUM") as psL:
            for pas in range(2):       # dt 0-7, then 8-15
                o_ps = [psL.tile([128, TOKC], F32, tag=f"o_ps{i}",
                                 name=f"o_ps{i}") for i in range(8)]
                for grp in range(4):
                    for hf in range(2):   # half-loads: first matmuls start
                        wl = pL.tile([128, 8, 1024], BF16, tag="wl",
                                     name="wl")
                        nc.sync.dma_start(
                            wl[:], wlin[pas, grp][:, hf * 8:(hf + 1) * 8, :])
                        for fl in range(8):
                            f = grp * 16 + hf * 8 + fl
                            for i in range(8):
                                nc.tensor.matmul(
                                    o_ps[i][:],
                                    wl[:, fl, i * 128:(i + 1) * 128],
                                    act[f][:],
                                    start=(f == 0),
                                    stop=(f == F_TILES - 1))
                for i in range(8):
                    dt = pas * 8 + i
                    ob = pOb.tile([128, TOKC], F32, tag="ob", name="ob")
                    nc.vector.tensor_add(ob[:], o_ps[i][:], res[dt][:])
                    nc.sync.dma_start(outT[dt], ob[:])


# ---------------------------------------------------------------------------
# Host side
# ---------------------------------------------------------------------------
def make_host_inputs(x, positions, w_q, w_kv, w_attn_vec, scale_pre_attn,
                     scale_pre_ffw, w_gating, w_linear):
    """Build the per-core input maps (all numpy, pre-tiled partition-major)."""
    x = np.asarray(x, np.float32)
    positions = np.asarray(positions)
    w_q = np.asarray(w_q, np.float32)
    w_kv = np.asarray(w_kv, np.float32)
    w_attn_vec = np.asarray(w_attn_vec, np.float32)
    s1 = 1.0 + np.asarray(scale_pre_attn, np.float32)
    s2 = 1.0 + np.asarray(scale_pre_ffw, np.float32)
    w_gating = np.asarray(w_gating, np.float32)
    w_linear = np.asarray(w_linear, np.float32)

    xT = np.ascontiguousarray(x.reshape(BT, D).T)          # [D, BT] f32
    # xt[c, p, kt, n] = xT[kt*128+p, c*512+n]
    xt = np.ascontiguousarray(
        xT.reshape(DT_TILES, 128, NCHUNK, TOKC).transpose(2, 1, 0, 3)
    ).astype(NP_BF16)

    pos = positions.reshape(BT).astype(np.float32)         # [BT]
    half = H // 2
    timescale = (10000.0 ** ((2.0 / H) * np.arange(half, dtype=np.float32)))
    rad = pos[None, :] / timescale[:, None]                # [64, BT]
    costab = np.cos(rad).astype(np.float32)
    sintab = np.sin(rad).astype(np.float32)

    # wg[fs, p, kt, m]: m 0:512 = gate cols, 512:1024 = up cols (s2 folded)
    wg0 = (w_gating[0] * s2[:, None]).astype(NP_BF16)      # [D, F]
    wg1 = (w_gating[1] * s2[:, None]).astype(NP_BF16)
    wg0_t = wg0.reshape(DT_TILES, 128, F // 512, 512).transpose(2, 1, 0, 3)
    wg1_t = wg1.reshape(DT_TILES, 128, F // 512, 512).transpose(2, 1, 0, 3)
    wg_p = np.ascontiguousarray(
        np.concatenate([wg0_t, wg1_t], axis=3))            # [fs, p, kt, 1024]

    wlin_bf = w_linear.astype(NP_BF16)                     # [F, D]
    # wlin[pas, grp, p, fl, m] = w_linear[(grp*16+fl)*128 + p, pas*1024 + m]
    wl_t = wlin_bf.reshape(4, 16, 128, 2, 1024)
    wlin_p = np.ascontiguousarray(wl_t.transpose(3, 0, 2, 1, 4))

    # wav[p, kt, d]: row block kt = q head kt (H=128 rows per head)
    wav_full = w_attn_vec.reshape(N * H, D).astype(NP_BF16)
    wav_p = np.ascontiguousarray(
        wav_full.reshape(DT_TILES, 128, D).transpose(1, 0, 2))

    in_maps = []
    for c in range(NCORES):
        hq0, hq1 = 2 * c, 2 * c + 1
        wq0 = w_q[hq0] * s1[:, None] * (H ** -0.5)
        wq1 = w_q[hq1] * s1[:, None] * (H ** -0.5)
        wk = w_kv[0, c] * s1[:, None]
        wv = w_kv[1, c] * s1[:, None]
        wqkv_c = np.concatenate([wq0, wq1, wk, wv], axis=1).astype(NP_BF16)
        wqkv_p = np.ascontiguousarray(
            wqkv_c.reshape(DT_TILES, 128, 4 * H).transpose(1, 0, 2))
        xs_c = np.ascontiguousarray(
            xT[:, c * TOKC:(c + 1) * TOKC]
            .reshape(DT_TILES, 128, TOKC).transpose(1, 0, 2))
        in_maps.append({
            "xt": xt,
            "xs": xs_c,
            "wqkv": wqkv_p,
            "wav": wav_p,
            "wg": wg_p,
            "wlin": wlin_p,
            "costab": costab,
            "sintab": sintab,
        })
    return in_maps


def assemble_output(results):
    """results: list of per-core {"outT": [DT, 128, TOKC] f32} -> [B,T,D]."""
    out = np.empty((BT, D), np.float32)
    for c, r in enumerate(results):
        o = np.asarray(r["outT"])                          # [16, 128, 512]
        out[c * TOKC:(c + 1) * TOKC, :] = o.reshape(D, TOKC).T
    return out.reshape(B, T, D)


_CACHE = {}


def _get_program():
    if "nc" not in _CACHE:
        _CACHE["nc"] = build_program(NCORES)
    return _CACHE["nc"]


def kernel(x, positions, attn_mask, w_q, w_kv, w_attn_vec, scale_pre_attn,
           scale_pre_ffw, w_gating, w_linear):
    nc = _get_program()
    in_maps = make_host_inputs(x, positions, w_q, w_kv, w_attn_vec,
                               scale_pre_attn, scale_pre_ffw, w_gating,
                               w_linear)
    _CACHE["in_maps"] = in_maps
    res = run_bass_kernel_spmd(nc, in_maps, list(range(NCORES)))
    return assemble_output(res.results)
